# revision 2
# baseline (speedup 1.0000x reference)
"""GCN (2-layer + MLP head) on 8 Trainium2 NeuronCores — gather+reduce design.

Per layer: sharded GEMM -> fp32 table slice -> AllGather -> dst-grouped
rectangle gathers (4 SWDGE queues, -1 skip pads, pre-zeroed staging) ->
DVE mid-axis tensor_reduce per (dst-tile, src-chunk) rectangle -> SBUF agg
-> pointwise. No DMA scatter-add: fp32 accumulation on the Vector engine
(also avoids the CCE reduced-precision accumulate of the scatter path).

Host does index math only. The rectangle structure (heights, segment
packing, per-call valid counts) is common across cores (SPMD); per-core
variation lives in the idx data (-1 = skip, ZROW = gather a zero row).
"""
import os

import numpy as np

import concourse.bacc as bacc
import concourse.mybir as mybir
from concourse.tile import TileContext
from concourse.bass_utils import run_bass_kernel_spmd

N = 100000
NS_RAW = 12500
NS = 12544              # 98 * 128
NTILE = NS // 128       # 98
N8 = NS * 8             # 100352
CHUNK = N8 // 4         # 25088
NCHUNK = 4
IN_CH, HID, HID2, OUT = 256, 64, 32, 2
ZROW = 12500            # zero row within every chunk (even slice's pad rows)
CBLK = 8                # blocks per gather call (1024 tokens)
SEGBLK = 64             # blocks per staging segment (8 calls)

_compiled = {}


def _build_schedule(src, dst):
    """Common rectangle schedule + per-core idx streams.

    Returns (idx16 [8, TOKS] int16, regs per call, segs, nseg).
    """
    core = dst // NS_RAW
    zr_rows_g = np.concatenate([np.arange(12500, 12544), np.arange(25044, 25088)])
    rng_g = np.random.default_rng(777)
    dl = dst - NS_RAW * core
    tau = dl // 128
    p = dl % 128
    tidx = (src // NS_RAW) * NS + (src % NS_RAW)
    k = tidx // CHUNK
    li = tidx % CHUNK

    # occurrence rank within (core, k, tau, p)
    key = ((core * NCHUNK + k) * NTILE + tau) * 128 + p
    order = np.argsort(key, kind="stable")
    ks = key[order]
    first = np.r_[True, ks[1:] != ks[:-1]]
    gs = np.where(first, np.arange(len(ks)), 0)
    np.maximum.accumulate(gs, out=gs)
    rank_s = np.arange(len(ks)) - gs
    rank = np.empty(len(ks), np.int64)
    rank[order] = rank_s

    # quantile heights: h*(k, tau) chosen so only ~SPQ of the 1024
    # (core, partition) columns overflow; overflow edges go to the spill
    # path (gather + dma_scatter_add), which is cheap for small counts.
    SPQ = int(os.environ.get("SPQ", "21"))
    kt_all = k * NTILE + tau
    cnt = np.zeros((NCHUNK * NTILE, 8 * 128), np.int64)
    key2 = kt_all * 1024 + core * 128 + p
    np.add.at(cnt.reshape(-1), key2, 1)
    h = np.zeros(NCHUNK * NTILE, np.int64)
    for i in range(NCHUNK * NTILE):
        ci_ = cnt[i]
        hi = int(ci_.max())
        if hi == 0:
            continue
        hh = hi
        while hh > 1 and (ci_ > hh - 1).sum() <= SPQ:
            hh -= 1
        h[i] = hh

    # pack rectangles into segments (chunk-major, tau ascending)
    segs = []
    seg_of = np.zeros(NCHUNK * NTILE, np.int64)
    boff_of = np.zeros(NCHUNK * NTILE, np.int64)
    cur_chunk, cur_blocks, cur_rects = -1, 0, []
    for kk in range(NCHUNK):
        for tt in range(NTILE):
            hh = int(h[kk * NTILE + tt])
            if hh == 0:
                continue
            if cur_chunk != kk or cur_blocks + hh > SEGBLK:
                if cur_rects:
                    segs.append((cur_chunk, cur_rects))
                cur_chunk, cur_blocks, cur_rects = kk, 0, []
            seg_of[kk * NTILE + tt] = len(segs)
            boff_of[kk * NTILE + tt] = cur_blocks
            cur_rects.append((tt, cur_blocks, hh))
            cur_blocks += hh
    if cur_rects:
        segs.append((cur_chunk, cur_rects))
    nseg = len(segs)
    TOKS = nseg * SEGBLK * 128

    idx16 = np.full((8, TOKS), -1, np.int64)
    kt = k * NTILE + tau
    main = rank < h[kt]
    slot = (seg_of[kt[main]] * SEGBLK + boff_of[kt[main]] + rank[main]) * 128 + p[main]
    idx16[core[main], slot] = li[main]

    # ---- spill streams: per chunk, rounds of unique dsts ----
    sp = ~main
    sp_core, sp_k, sp_li = core[sp], k[sp], li[sp]
    sp_dl = dl[sp]
    sp_round = rank[sp] - h[kt[sp]]
    spill_calls = []            # list of (chunk, ntok)
    sp_src = [[] for _ in range(8)]
    sp_dst = [[] for _ in range(8)]
    TRASH = NS - 1
    for kk in range(NCHUNK):
        mk = sp_k == kk
        if not mk.any():
            continue
        rmax = int(sp_round[mk].max())
        for r in range(rmax + 1):
            mr = mk & (sp_round == r)
            ntok = 0
            percore = []
            for c in range(8):
                mc = mr & (sp_core == c)
                percore.append((sp_li[mc], sp_dl[mc]))
                ntok = max(ntok, int(mc.sum()))
            ntok = -(-ntok // 128) * 128
            if ntok == 0:
                continue
            # a round may exceed 1024: split into calls of <=1024
            offs = list(range(0, ntok, 1024))
            for o in offs:
                n = min(1024, ntok - o)
                spill_calls.append((kk, n))
            for c in range(8):
                sl, dd = percore[c]
                ns_ = len(sl)
                pad = ntok - ns_
                sp_src[c].append(np.r_[sl, zr_rows_g[rng_g.integers(0, 88, pad)]])
                sp_dst[c].append(np.r_[dd, np.full(pad, TRASH, np.int64)])
    if spill_calls:
        sp_src16 = np.stack([np.concatenate(s) for s in sp_src])
        sp_dst16 = np.stack([np.concatenate(s) for s in sp_dst])
    else:
        sp_src16 = np.zeros((8, 0), np.int64)
        sp_dst16 = np.zeros((8, 0), np.int64)

    zr_rows = np.concatenate([np.arange(12500, 12544), np.arange(25044, 25088)])
    rngp = np.random.default_rng(12345)
    if not os.environ.get("V2_ZROWPAD"):
        # every 128-token block needs >=1 valid idx per core (all-(-1) blocks
        # appear to break the DGE ucode); point one slot at a spread zero row
        ib = idx16.reshape(8, nseg * SEGBLK, 128)
        empty = (ib < 0).all(axis=2)          # [8, nblk]
        ec, eb = np.nonzero(empty)
        ib[ec, eb, rngp.integers(0, 128, len(ec))] = zr_rows[rngp.integers(0, 88, len(ec))]

    # trailing -1s after a call's last valid get garbage-row writes on HW
    # (mid-stream -1s skip correctly) -> force the last slot of every call
    # to be a valid zero-row gather on every core
    ncall0 = nseg * SEGBLK // CBLK
    iv0 = idx16.reshape(8, ncall0, CBLK * 128)
    iv0[:, :, -1] = np.where(iv0[:, :, -1] < 0,
                             zr_rows[rngp.integers(0, 88, (8, ncall0))],
                             iv0[:, :, -1])

    # per-call valid-count equalization (reg baked into the program)
    ncall = nseg * SEGBLK // CBLK
    iv = idx16.reshape(8, ncall, CBLK * 128)
    vc = (iv >= 0).sum(axis=2)                      # [8, ncall]
    regs = vc.max(axis=0)                           # [ncall]
    for ci in range(ncall):
        v = regs[ci]
        if v == 0:
            continue
        for c in range(8):
            need = v - vc[c, ci]
            if need > 0:
                neg = np.nonzero(iv[c, ci] < 0)[0][:need]
                iv[c, ci, neg] = zr_rows[rngp.integers(0, 88, need)]
    if True:
        # spread pad reads across all 88 zero rows of each chunk (even slice
        # rows 12500..12543, odd slice rows 25044..25087) to avoid an HBM
        # bank hotspot from hammering a single 256B row
        npad = (idx16 < 0).sum()
        zr = np.concatenate([np.arange(12500, 12544), np.arange(25044, 25088)])
        rngp = np.random.default_rng(12345)
        idx16[idx16 < 0] = zr[rngp.integers(0, 88, npad)]
        regs = np.full_like(regs, CBLK * 128)
    return (idx16.astype(np.int16), [int(r) for r in regs], segs, nseg,
            sp_src16.astype(np.int16), sp_dst16.astype(np.int16), spill_calls)


def _wrap16(a):
    w = a.reshape(-1, 16).T.copy()
    return np.tile(w, (8, 1))


def _build_program(segs, nseg, regs, spill_calls, sp_tok):
    nc = bacc.Bacc(None, target_bir_lowering=False,
                   dynamic_dma_scratch_size=49152,
                   num_swdge_queues=int(os.environ.get("NQ", "4")))
    dt = mybir.dt
    P = nc.declare_dram_parameter
    xT = P("xT", [IN_CH, NS], dt.float32, isOutput=False)
    w1p = P("w1p", [128, 128], dt.float32, isOutput=False)
    w2 = P("w2", [HID, HID], dt.float32, isOutput=False)
    wh1 = P("wh1", [HID, HID2], dt.float32, isOutput=False)
    wh2 = P("wh2", [HID2, OUT], dt.float32, isOutput=False)
    b1f = P("b1f", [128, HID], dt.float32, isOutput=False)
    b2f = P("b2f", [128, HID], dt.float32, isOutput=False)
    bh1 = P("bh1", [HID2, 1], dt.float32, isOutput=False)
    bh2 = P("bh2", [OUT, 1], dt.float32, isOutput=False)
    dinvP = P("dinvP", [128, NTILE], dt.float32, isOutput=False)
    idxP = P("idx16", [128, nseg * SEGBLK * 8], dt.int16, isOutput=False)
    spsP = P("sps16", [128, max(sp_tok // 16, 8)], dt.int16, isOutput=False)
    spdP = P("spd16", [128, max(sp_tok // 16, 8)], dt.int16, isOutput=False)
    outT = P("outT", [OUT, NS], dt.float32, isOutput=True)

    slice_d = [nc.dram_tensor(f"slice{l}", [NS, HID], dt.float32) for l in (1, 2)]
    spagg_d = [nc.dram_tensor(f"spagg{l}", [NS, HID], dt.float32) for l in (1, 2)]
    table_d = [nc.dram_tensor(f"table{l}", [N8, HID], dt.float32) for l in (1, 2)]

    from concourse.masks import make_identity
    add = mybir.AluOpType.add
    relu = mybir.ActivationFunctionType.Relu
    NQ = int(os.environ.get("NQ", "4"))
    qctr = [0]
    HTN = NTILE // 2

    with TileContext(nc) as tc:
        with tc.tile_pool(name="const", bufs=1) as cp, \
             tc.tile_pool(name="work", bufs=4) as wp, \
             tc.tile_pool(name="stg", bufs=4) as sp, \
             tc.tile_pool(name="idx", bufs=5) as ip, \
             tc.tile_pool(name="hT", bufs=1) as hp, \
             tc.tile_pool(name="ps", bufs=2, space="PSUM") as pp:
            w1sb = cp.tile([128, 128], dt.float32)
            nc.sync.dma_start(out=w1sb[:], in_=w1p[:])
            w2sb = cp.tile([HID, HID], dt.float32)
            nc.sync.dma_start(out=w2sb[:], in_=w2[:])
            wh1sb = cp.tile([HID, HID2], dt.float32)
            nc.sync.dma_start(out=wh1sb[:], in_=wh1[:])
            wh2sb = cp.tile([HID2, OUT], dt.float32)
            nc.sync.dma_start(out=wh2sb[:], in_=wh2[:])
            b1sb = cp.tile([128, HID], dt.float32)
            nc.sync.dma_start(out=b1sb[:], in_=b1f[:])
            b2sb = cp.tile([128, HID], dt.float32)
            nc.sync.dma_start(out=b2sb[:], in_=b2f[:])
            bh1sb = cp.tile([HID2, 1], dt.float32)
            nc.sync.dma_start(out=bh1sb[:], in_=bh1[:])
            bh2sb = cp.tile([OUT, 1], dt.float32)
            nc.sync.dma_start(out=bh2sb[:], in_=bh2[:])
            dsb = cp.tile([128, NTILE], dt.float32)
            nc.sync.dma_start(out=dsb[:], in_=dinvP[:])
            ident = cp.tile([128, 128], dt.float32)
            make_identity(nc, ident[:])

            agg = cp.tile([128, NTILE * HID], dt.float32)
            hwsb = cp.tile([128, NTILE * HID], dt.float32)
            sps_sb = cp.tile([128, max(sp_tok // 16, 8)], dt.int16)
            nc.sync.dma_start(out=sps_sb[:], in_=spsP[:])
            spd_sb = cp.tile([128, max(sp_tok // 16, 8)], dt.int16)
            nc.sync.dma_start(out=spd_sb[:], in_=spdP[:])
            zt = cp.tile([128, 512], dt.float32)
            nc.vector.memset(zt[:], 0.0)

            def zero_spagg(spagg):
                a3 = spagg[:].rearrange("(a p) d -> p a d", p=128)
                z3 = zt[:].rearrange("p (a d) -> p a d", d=HID)
                a0 = 0
                while a0 < NTILE:
                    an = min(8, NTILE - a0)
                    nc.sync.dma_start(out=a3[:, a0:a0 + an, :], in_=z3[:, 0:an, :])
                    a0 += an

            def gemm_store(m, ps, sl):
                hw = hwsb[:, m * HID:(m + 1) * HID]
                nc.vector.tensor_scalar_mul(hw, ps[:], dsb[:, m:m + 1])
                nc.sync.dma_start(out=sl[m * 128:(m + 1) * 128, :], in_=hw)

            def aggregate(layer):
                table = table_d[layer]
                nc.vector.memset(agg[:], 0.0)
                if os.environ.get("V2_NOAGG"):
                    return
                for si, (kk, rects) in enumerate(segs):
                    stg = sp.tile([128, SEGBLK * HID], dt.float32, tag="stg")
                    it = ip.tile([128, SEGBLK * 8], dt.int16, tag="it")
                    nc.sync.dma_start(
                        out=it[:], in_=idxP[:, si * SEGBLK * 8:(si + 1) * SEGBLK * 8])
                    for cb in range(0, SEGBLK, CBLK):
                        ci = si * (SEGBLK // CBLK) + cb // CBLK
                        if regs[ci] == 0:
                            continue
                        g3 = stg[:, cb * HID:(cb + CBLK) * HID].rearrange(
                            "p (c d) -> p c d", d=HID)
                        nc.gpsimd.dma_gather(
                            g3, table[kk * CHUNK:(kk + 1) * CHUNK, :],
                            it[:, cb * 8:(cb + CBLK) * 8],
                            CBLK * 128, regs[ci], HID,
                            single_packet=not os.environ.get("V2_SP_OFF"),
                            queue_num=qctr[0] % NQ)
                        qctr[0] += 1
                    for (tt, bo, hh) in (() if os.environ.get("V2_NORED") else rects):
                        r = wp.tile([128, HID], dt.float32, tag="r")
                        gT = stg[:, bo * HID:(bo + hh) * HID].rearrange(
                            "p (b d) -> p d b", d=HID)
                        nc.vector.tensor_reduce(r[:], gT, mybir.AxisListType.X, add)
                        a = agg[:, tt * HID:(tt + 1) * HID]
                        nc.vector.tensor_add(a, a, r[:])
                # ---- spill path: gather + dma_scatter_add into spagg ----
                o16 = 0
                for (kk, ntok) in spill_calls:
                    gt = wp.tile([128, (ntok // 128) * HID], dt.float32, tag="spg")
                    g3 = gt[:].rearrange("p (c d) -> p c d", d=HID)
                    nc.gpsimd.dma_gather(
                        g3, table[kk * CHUNK:(kk + 1) * CHUNK, :],
                        sps_sb[:, o16:o16 + ntok // 16], ntok, ntok, HID,
                        queue_num=qctr[0] % NQ)
                    qctr[0] += 1
                    nc.gpsimd.dma_scatter_add(
                        spagg_d[layer][:], g3,
                        spd_sb[:, o16:o16 + ntok // 16], ntok, ntok, HID,
                        queue_num=qctr[0] % NQ)
                    qctr[0] += 1
                    o16 += ntok // 16

            def pointwise_t(bsb, layer):
                for half in range(2):
                    mlo = half * HTN
                    hT = hp.tile([HID, HTN * 128], dt.float32, tag="hT")
                    for mi in range(HTN):
                        m = mlo + mi
                        spt = wp.tile([128, HID], dt.float32, tag="spt")
                        nc.sync.dma_start(
                            out=spt[:], in_=spagg_d[layer][m * 128:(m + 1) * 128, :])
                        s = wp.tile([128, HID], dt.float32, tag="s")
                        nc.vector.tensor_add(s[:], agg[:, m * HID:(m + 1) * HID],
                                             hwsb[:, m * HID:(m + 1) * HID])
                        nc.vector.tensor_add(s[:], s[:], spt[:])
                        nc.vector.tensor_scalar_mul(s[:], s[:], dsb[:, m:m + 1])
                        nc.vector.tensor_add(s[:], s[:], bsb[:])
                        hh = wp.tile([128, HID], dt.float32, tag="hh")
                        nc.scalar.activation(hh[:], s[:], relu)
                        pt = pp.tile([HID, 128], dt.float32, tag="pt")
                        nc.tensor.transpose(pt[:], hh[:], ident[:])
                        nc.vector.tensor_copy(hT[:, mi * 128:(mi + 1) * 128], pt[:])
                    yield half, mlo, hT

            # ---- layer 1 GEMM ----
            for m in range(NTILE):
                mc = slice(m * 128, (m + 1) * 128)
                xa = wp.tile([128, 128], dt.float32, tag="xa")
                nc.sync.dma_start(out=xa[:], in_=xT[0:128, mc])
                xb = wp.tile([128, 128], dt.float32, tag="xb")
                nc.sync.dma_start(out=xb[:], in_=xT[128:256, mc])
                ps = pp.tile([128, HID], dt.float32, tag="ps")
                nc.tensor.matmul(ps[:], xa[:], w1sb[:, 0:HID], start=True, stop=False)
                nc.tensor.matmul(ps[:], xb[:], w1sb[:, HID:128], start=False, stop=True)
                gemm_store(m, ps, slice_d[0])
            zero_spagg(spagg_d[0])
            nc.gpsimd.collective_compute(
                "AllGather", mybir.AluOpType.bypass,
                replica_groups=[list(range(8))],
                ins=[slice_d[0][:]], outs=[table_d[0][:]])
            aggregate(0)

            # ---- pointwise 1 + layer 2 GEMM ----
            for half, mlo, hT in pointwise_t(b1sb, 0):
                for mi in range(HTN):
                    m = mlo + mi
                    ps = pp.tile([128, HID], dt.float32, tag="ps")
                    nc.tensor.matmul(ps[:], hT[:, mi * 128:(mi + 1) * 128], w2sb[:],
                                     start=True, stop=True)
                    gemm_store(m, ps, slice_d[1])
            zero_spagg(spagg_d[1])
            nc.gpsimd.collective_compute(
                "AllGather", mybir.AluOpType.bypass,
                replica_groups=[list(range(8))],
                ins=[slice_d[1][:]], outs=[table_d[1][:]])
            aggregate(1)

            # ---- pointwise 2 + head ----
            for half, mlo, hT in pointwise_t(b2sb, 1):
                for n0 in range(0, HTN * 128, 448):
                    ncol = slice(n0, n0 + 448)
                    gcol = slice(mlo * 128 + n0, mlo * 128 + n0 + 448)
                    pz = pp.tile([HID2, 448], dt.float32, tag="pz")
                    nc.tensor.matmul(pz[:], wh1sb[:], hT[:, ncol], start=True, stop=True)
                    zb = wp.tile([HID2, 448], dt.float32, tag="zb")
                    nc.scalar.activation(zb[:], pz[:], relu, bias=bh1sb[:])
                    po = pp.tile([OUT, 448], dt.float32, tag="po")
                    nc.tensor.matmul(po[:], wh2sb[:], zb[:], start=True, stop=True)
                    ob = wp.tile([OUT, 448], dt.float32, tag="ob")
                    nc.vector.tensor_scalar_add(ob[:], po[:], bh2sb[:])
                    nc.sync.dma_start(out=outT[:, gcol], in_=ob[:])

    nc.finalize()
    return nc


def kernel(x, edge_index, W1, b1, W2, b2, Wh1, bh1, Wh2, bh2, _trace=False):
    x = np.asarray(x, np.float32)
    src = np.asarray(edge_index[0], np.int64)
    dst = np.asarray(edge_index[1], np.int64)

    idx16, regs, segs, nseg, sp_src16, sp_dst16, spill_calls = \
        _build_schedule(src, dst)
    sp_tok = sp_src16.shape[1]
    sig = (nseg, tuple(regs), tuple(spill_calls),
           tuple((kk, tuple(rects)) for kk, rects in segs))
    if sig not in _compiled:
        _compiled[sig] = _build_program(segs, nseg, regs, spill_calls, sp_tok)
    nc = _compiled[sig]

    deg = np.bincount(dst, minlength=N).astype(np.float64) + 1.0
    dinv = (1.0 / np.sqrt(deg)).astype(np.float32)

    W1 = np.asarray(W1, np.float32)
    w1p = np.concatenate([W1[:128], W1[128:]], axis=1)
    b1f = np.tile(np.asarray(b1, np.float32)[None, :], (128, 1))
    b2f = np.tile(np.asarray(b2, np.float32)[None, :], (128, 1))
    bh1c = np.asarray(bh1, np.float32)[:, None]
    bh2c = np.asarray(bh2, np.float32)[:, None]

    in_maps = []
    for c in range(8):
        xs = np.zeros((NS, IN_CH), np.float32)
        xs[:NS_RAW] = x[c * NS_RAW:(c + 1) * NS_RAW]
        dv = np.zeros(NS, np.float32)   # pad rows: dinv=0 zeroes table pads
        dv[:NS_RAW] = dinv[c * NS_RAW:(c + 1) * NS_RAW]
        in_maps.append({
            "xT": np.ascontiguousarray(xs.T),
            "w1p": np.ascontiguousarray(w1p),
            "w2": np.asarray(W2, np.float32),
            "wh1": np.asarray(Wh1, np.float32),
            "wh2": np.asarray(Wh2, np.float32),
            "b1f": b1f, "b2f": b2f, "bh1": bh1c, "bh2": bh2c,
            "dinvP": np.ascontiguousarray(dv.reshape(NTILE, 128).T),
            "idx16": _wrap16(idx16[c]),
            "sps16": (_wrap16(sp_src16[c]) if sp_tok else
                      np.zeros((128, 8), np.int16)),
            "spd16": (_wrap16(sp_dst16[c]) if sp_tok else
                      np.zeros((128, 8), np.int16)),
        })

    res = run_bass_kernel_spmd(nc, in_maps, list(range(8)), trace=_trace)
    out = np.empty((N, OUT), np.float32)
    for c in range(8):
        out[c * NS_RAW:(c + 1) * NS_RAW] = res.results[c]["outT"].T[:NS_RAW]
    if _trace:
        kernel.last_results = res
    return out
